# revision 27
# baseline (speedup 1.0000x reference)
"""Multi-head attention (B=2, S=4096, D=512, H=8) on 8 Trainium2 NeuronCores.

Sharding: core c handles batch b = c//4 and q-rows [1024*(c%4), 1024*(c%4+1)).
No collectives.

Schedule (v2): the Scalar engine's softmax-exp (33.6M elem/core at 1
elem/lane/cycle @1.2GHz ~ 220us busy) is the hard floor, so everything else
is organized to hide under it:
  - head-PAIR processing: the two heads of a pair occupy partition halves
    0-63 / 64-127; their K=64 score matmuls are issued back-to-back with
    tile_position (0,0)/(64,0) so they run CONCURRENTLY in the two row
    halves of the PE array (2x effective throughput, no warmkeepers --
    the interleaved K=128 AV/projection matmuls keep the HAM clock warm).
  - projections for pair p+1 and the AV/finalize work of the previous
    (pair,qc) are chopped into small "quanta" and drained into the score
    group loop, so the tensor engine works while the scalar engine exps.
    Quanta carry a deadline (attention index) and are force-drained
    before the attention that needs their results / their ring slots.
  - PSUM map (8 banks): scores 2x[128,2,512] (4) + av 2x[128,4,128] (2,
    one per head, qt accumulation groups sequential within the bank as
    required by the 2KB zero-region rule) + proj/transpose ring
    2x[128,512] (2).
  - output transposes are col-tiled pairs (0,0)/(0,64) into one [128,128]
    PSUM tile; final projection contracts head pairs as row-tiled
    concurrent matmuls into two separate PSUM tiles (summed on DVE).

Numerics identical to v1: bf16 matmul operands, fp32 PSUM accumulation,
softmax exp in fp32->bf16 on Scalar (no max-subtraction; scores ~N(0,1)),
denominator via ones-column in the attn@V matmul, normalization on DVE.
"""

import numpy as np
import ml_dtypes

import concourse.bass as bass
import concourse.tile as tile
import concourse.mybir as mybir
from concourse import bacc
from concourse.bass_utils import run_bass_kernel_spmd
from concourse.masks import make_identity

BF16 = ml_dtypes.bfloat16
F32 = mybir.dt.float32
BF = mybir.dt.bfloat16
EXP = mybir.ActivationFunctionType.Exp

N_CORES = 8
B, S, D = 2, 4096, 512
H, DEP = 8, 64
SQ = S // 4            # q rows per core
N_QT = SQ // 128       # q 128-tiles per core (8)
N_KT = S // 128        # k 128-tiles (32)
N_DC = D // 128        # 128-chunks of d_model (4)
N_PAIR = 4             # head pairs; pair p = heads 2p, 2p+1
G = 2                  # k-tiles per score PSUM tile / exp instruction

_COMPILED = None
_WARMED = False


def build_kernel(with_bias=True):
    nc = bacc.Bacc("TRN2", target_bir_lowering=False, debug=False,
                   num_devices=N_CORES)

    # ---- I/O ----
    qT = nc.dram_tensor("qT", [D, SQ], BF, kind="ExternalInput")
    kT = nc.dram_tensor("kT", [D, S], BF, kind="ExternalInput")
    vT = nc.dram_tensor("vT", [D, S], BF, kind="ExternalInput")
    w_in, b_in = {}, {}
    for name in ("wq", "wk", "wv"):
        w_in[name] = nc.dram_tensor(name, [D, D], BF, kind="ExternalInput")
    # wo host-reorganized as [128, N_PAIR, D]: rows 0:64 = head 2p's 64
    # contraction dims, rows 64:128 = head 2p+1's
    wo_in = nc.dram_tensor("wo", [128, N_PAIR, D], BF, kind="ExternalInput")
    for name in ("bq", "bk", "bv", "bo"):
        b_in[name] = nc.dram_tensor(name, [1, D], BF, kind="ExternalInput")
    out = nc.dram_tensor("out", [SQ, D], F32, kind="ExternalOutput")

    with tile.TileContext(nc) as tc:
        with (
            tc.tile_pool(name="const", bufs=1) as cpool,
            tc.tile_pool(name="hring", bufs=2) as hpool,
            tc.tile_pool(name="atp", bufs=2) as atpool,
            tc.tile_pool(name="xin", bufs=2) as xpool,
            tc.tile_pool(name="small", bufs=4) as spool,
            tc.tile_pool(name="big", bufs=1) as bigpool,
            tc.tile_pool(name="scores", bufs=2, space="PSUM") as scpool,
            tc.tile_pool(name="avps", bufs=2, space="PSUM") as avpool,
            tc.tile_pool(name="projps", bufs=2, space="PSUM") as pspool,
        ):
            # ---- constants ----
            ident = cpool.tile([128, 128], BF, name="ident")
            make_identity(nc, ident)
            scratch = cpool.tile([128, 512], BF, name="scratch")
            nc.gpsimd.memset(scratch, 0.25)

            wsb, bsb = {}, {}
            for name in ("wq", "wk", "wv"):
                t = cpool.tile([128, N_DC, D], BF, name=f"w_{name}")
                for c in range(N_DC):
                    nc.sync.dma_start(t[:, c, :],
                                      w_in[name][c * 128:(c + 1) * 128, :])
                wsb[name] = t
            wosb = cpool.tile([128, N_PAIR, D], BF, name="w_wo")
            nc.sync.dma_start(wosb, wo_in[:])
            if with_bias:
                ones = cpool.tile([1, 512], BF, name="ones")
                nc.gpsimd.memset(ones, 1.0)
                btile = cpool.tile([1, 4, D], BF, name="biases")
                for i, name in enumerate(("bq", "bk", "bv", "bo")):
                    nc.sync.dma_start(btile[:, i, :], b_in[name][:])
                    bsb[name] = btile[:, i, :]

            # transposed per-head attn outputs, heads paired on partition
            # halves: otr2[64*hi:64*(hi+1), qt, p, :] = head (2p+hi)
            otr2 = bigpool.tile([128, N_QT, N_PAIR, 128], BF, name="otr2")

            # ---- deferred work queue: (deadline_att_idx, cost, fn) ----
            deferred = []
            oh_live = {}

            def drain_budget(budget):
                spent = 0
                while deferred and spent < budget:
                    _, cost, fn = deferred.pop(0)
                    fn()
                    spent += cost

            def drain_due(att_idx):
                while deferred and deferred[0][0] <= att_idx:
                    _, _, fn = deferred.pop(0)
                    fn()

            # ---- projection quanta for pair p ----
            # each quantum is split into a DMA-issue part and an MM part;
            # the queue interleaves them two-deep (ring bufs=2) so the PE
            # stream never waits on an xin load mid-queue
            def qk_quantum(src, wname, bname, p, rc, dst):
                box = {}

                def dma_fn():
                    xin = xpool.tile([128, N_DC, 512], BF, tag="xin",
                                     name="xin")
                    for dc in range(N_DC):
                        nc.sync.dma_start(
                            xin[:, dc, :],
                            src[dc * 128:(dc + 1) * 128,
                                rc * 512:(rc + 1) * 512])
                    box["xin"] = xin

                def mm_fn():
                    xin = box["xin"]
                    ps = pspool.tile([128, 512], F32, tag="ps", name="ps")
                    for dc in range(N_DC):
                        nc.tensor.matmul(
                            ps, wsb[wname][:, dc, p * 128:(p + 1) * 128],
                            xin[:, dc, :], start=(dc == 0),
                            stop=(not with_bias and dc == N_DC - 1))
                    if with_bias:
                        nc.tensor.matmul(
                            ps, bsb[bname][0:1, p * 128:(p + 1) * 128],
                            ones[0:1, :], start=False, stop=True)
                    nc.vector.tensor_copy(dst[:, rc * 512:(rc + 1) * 512],
                                          ps)
                return (2300, dma_fn, mm_fn)

            def v_quantum(p, rc, half, vh_t):
                # projects seq rows [rc*512 + half*256, +256) = 2 k-tiles,
                # one PSUM accumulation group spanning both 128-col slices
                box = {}
                c0 = rc * 512 + half * 256

                def dma_fn():
                    xin = xpool.tile([128, N_DC, 256], BF, tag="xin",
                                     name="xinv")
                    for dc in range(N_DC):
                        nc.sync.dma_start(
                            xin[:, dc, :],
                            vT[dc * 128:(dc + 1) * 128, c0:c0 + 256])
                    box["xin"] = xin

                def mm_fn():
                    xin = box["xin"]
                    ps = pspool.tile([128, 512], F32, tag="ps", name="psv")
                    nmm = 2 * N_DC + (2 if with_bias else 0)
                    j = 0
                    for i in range(2):
                        for dc in range(N_DC):
                            nc.tensor.matmul(
                                ps[:, i * 128:(i + 1) * 128],
                                xin[:, dc, i * 128:(i + 1) * 128],
                                wsb["wv"][:, dc, p * 128:(p + 1) * 128],
                                start=(j == 0), stop=(j == nmm - 1))
                            j += 1
                        if with_bias:
                            nc.tensor.matmul(
                                ps[:, i * 128:(i + 1) * 128],
                                ones[0:1, 0:128],
                                bsb["bv"][0:1, p * 128:(p + 1) * 128],
                                start=False, stop=(j == nmm - 1))
                            j += 1
                    rt0 = c0 // 128
                    nc.vector.tensor_copy(
                        vh_t[:, rt0:rt0 + 2, :, 0:DEP],
                        ps[:, 0:256].rearrange("p (r h e) -> p r h e",
                                               r=2, h=2))
                return (1300, dma_fn, mm_fn)

            def q_quantum(p, rc, qhz_t):
                # like qk_quantum but writes the two zero-padded per-head
                # copies: qhz[:, hi, :] has the OTHER head's 64 partitions
                # zeroed so the score matmul can contract over the full
                # K=128 (feeding the HAM clock monitor) yet still compute
                # a single head's scores
                box = {}

                def dma_fn():
                    xin = xpool.tile([128, N_DC, 512], BF, tag="xin",
                                     name="xin")
                    for dc in range(N_DC):
                        nc.sync.dma_start(
                            xin[:, dc, :],
                            qT[dc * 128:(dc + 1) * 128,
                               rc * 512:(rc + 1) * 512])
                    box["xin"] = xin

                def mm_fn():
                    xin = box["xin"]
                    ps = pspool.tile([128, 512], F32, tag="ps", name="ps")
                    for dc in range(N_DC):
                        nc.tensor.matmul(
                            ps, wsb["wq"][:, dc, p * 128:(p + 1) * 128],
                            xin[:, dc, :], start=(dc == 0),
                            stop=(not with_bias and dc == N_DC - 1))
                    if with_bias:
                        nc.tensor.matmul(
                            ps, bsb["bq"][0:1, p * 128:(p + 1) * 128],
                            ones[0:1, :], start=False, stop=True)
                    sl = slice(rc * 512, (rc + 1) * 512)
                    nc.vector.tensor_copy(qhz_t[0:64, 0, sl], ps[0:64, :])
                    nc.vector.tensor_copy(qhz_t[64:128, 1, sl],
                                          ps[64:128, :])
                return (2300, dma_fn, mm_fn)

            def stagger(parts, deadline):
                # entry j runs mm_{j-1} then issues dma_j, so the PE work
                # of one quantum overlaps the next quantum's loads
                quanta = [(deadline, 100, parts[0][1])]
                for j in range(1, len(parts)):
                    dma_j, mm_prev = parts[j][1], parts[j - 1][2]

                    def entry(mm=mm_prev, dma=dma_j):
                        mm()
                        dma()
                    quanta.append((deadline, parts[j - 1][0], entry))
                quanta.append((deadline, parts[-1][0], parts[-1][2]))
                return quanta

            def make_pair_tiles(p, deadline, split_v=False):
                qhz_t = hpool.tile([128, 2, SQ], BF, tag="qh", name="qhz")
                kh_t = hpool.tile([128, S], BF, tag="kh", name="kh")
                vh_t = hpool.tile([128, N_KT, 2, DEP + 1], BF, tag="vh",
                                  name="vh")

                def memset_fn():
                    nc.gpsimd.memset(vh_t[:, :, :, DEP:DEP + 1], 1.0)
                    nc.gpsimd.memset(qhz_t[64:128, 0, :], 0.0)
                    nc.gpsimd.memset(qhz_t[0:64, 1, :], 0.0)
                if split_v:
                    # pair 0: q/k quanta run inline so attention can start
                    # early; v quanta drain during the first windows
                    qk_parts = [q_quantum(p, rc, qhz_t)
                                for rc in range(SQ // 512)]
                    qk_parts += [qk_quantum(kT, "wk", "bk", p, rc, kh_t)
                                 for rc in range(S // 512)]
                    v_parts = []
                    for rc in range(S // 512):
                        v_parts.append(v_quantum(p, rc, 0, vh_t))
                        v_parts.append(v_quantum(p, rc, 1, vh_t))
                    head = ([(deadline, 100, memset_fn)]
                            + stagger(qk_parts, deadline))
                    return qhz_t, kh_t, vh_t, head, stagger(v_parts,
                                                            deadline)
                parts = []
                for rc in range(SQ // 512):
                    parts.append(q_quantum(p, rc, qhz_t))
                for rc in range(S // 512):
                    parts.append(qk_quantum(kT, "wk", "bk", p, rc, kh_t))
                    parts.append(v_quantum(p, rc, 0, vh_t))
                    parts.append(v_quantum(p, rc, 1, vh_t))
                quanta = ([(deadline, 100, memset_fn)]
                          + stagger(parts, deadline))
                return qhz_t, kh_t, vh_t, quanta

            # ---- attention for (pair p, q-chunk qc) ----
            def attention(att_idx, p, qc, qhz_t, kh_t, vh_t):
                drain_due(att_idx)
                qsl = slice(qc * 512, (qc + 1) * 512)
                at0 = atpool.tile([128, N_KT, 512], BF, tag="at0",
                                  name="at0")
                at1 = atpool.tile([128, N_KT, 512], BF, tag="at1",
                                  name="at1")
                for g in range(N_KT // G):
                    sc0 = scpool.tile([128, G, 512], F32, tag="sc",
                                      name="sc0")
                    sc1 = scpool.tile([128, G, 512], F32, tag="sc",
                                      name="sc1")
                    # full K=128 contraction: the other head's partitions
                    # are zero in qhz, so each matmul computes one head's
                    # scores while feeding the HAM clock monitor
                    for i in range(G):
                        t = g * G + i
                        nc.tensor.matmul(
                            sc0[:, i, :],
                            kh_t[:, t * 128:(t + 1) * 128],
                            qhz_t[:, 0, qsl],
                            start=True, stop=True)
                        nc.tensor.matmul(
                            sc1[:, i, :],
                            kh_t[:, t * 128:(t + 1) * 128],
                            qhz_t[:, 1, qsl],
                            start=True, stop=True)
                    nc.scalar.activation(at0[:, g * G:(g + 1) * G, :], sc0,
                                         EXP, scale=0.125)
                    nc.scalar.activation(at1[:, g * G:(g + 1) * G, :], sc1,
                                         EXP, scale=0.125)
                    drain_budget(3400)
                return at0, at1

            # ---- AV + finalize quanta for (p, qc) ----
            def avfin_quantum(hi, qt, at_h, vh_t, av_h):
                def fn():
                    for t in range(N_KT):
                        nc.tensor.matmul(
                            av_h[:, qt, 0:DEP + 1],
                            at_h[:, t, qt * 128:(qt + 1) * 128],
                            vh_t[:, t, hi, :],
                            start=(t == 0), stop=(t == N_KT - 1))
                    if qt % 2 == 0:
                        # AV matmuls (N=65) don't feed the HAM clock
                        # monitor; keep it warm through AV-heavy stretches
                        keeper()
                    rec = spool.tile([128, 1], F32, tag="rec", name="rec",
                                     bufs=4)
                    nc.vector.reciprocal(rec, av_h[:, qt, DEP:DEP + 1])
                    oh = spool.tile([128, DEP], BF, tag=f"oh{hi}",
                                    name="oh", bufs=4)
                    nc.vector.tensor_scalar_mul(oh, av_h[:, qt, 0:DEP], rec)
                    oh_live[(hi, qt)] = oh
                return (2900, fn)

            def transpose_quantum(p, qc, qt):
                def fn():
                    qidx = qc * 4 + qt
                    trp = pspool.tile([128, 128], BF, tag="ps", name="trp")
                    nc.tensor.transpose(trp[0:64, :], oh_live.pop((0, qt)),
                                        ident, tile_position=(0, 0))
                    nc.tensor.transpose(trp[64:128, :], oh_live.pop((1, qt)),
                                        ident, tile_position=(0, 64))
                    nc.vector.tensor_copy(otr2[:, qidx, p, :], trp)
                return (400, fn)

            def enqueue_avfin(deadline, p, qc, at0, at1, vh_t):
                av0 = avpool.tile([128, 4, 128], F32, tag="av", name="av0")
                av1 = avpool.tile([128, 4, 128], F32, tag="av", name="av1")
                for hi, (at_h, av_h) in enumerate(((at0, av0), (at1, av1))):
                    for qt in range(4):
                        deferred.append(
                            (deadline,)
                            + avfin_quantum(hi, qt, at_h, vh_t, av_h))
                for qt in range(4):
                    deferred.append((deadline,) + transpose_quantum(p, qc, qt))

            # ---- final output projection for one q-tile ----
            # full K=128 contraction per pair sums both heads' (d=64)
            # contributions in one matmul
            def final_proj(qt):
                ps = pspool.tile([128, 512], F32, tag="ps", name="pso")
                for s in range(N_PAIR):
                    nc.tensor.matmul(
                        ps, otr2[:, qt, s, :], wosb[:, s, :],
                        start=(s == 0),
                        stop=(not with_bias and s == N_PAIR - 1))
                if with_bias:
                    nc.tensor.matmul(ps, ones[0:1, 0:128], bsb["bo"],
                                     start=False, stop=True)
                osb = spool.tile([128, 512], F32, tag="osb", name="osb",
                                 bufs=2)
                nc.vector.tensor_copy(osb, ps)
                nc.sync.dma_start(out[qt * 128:(qt + 1) * 128, :], osb)

            # ---- main flow ----
            def keeper():
                # HAM warm-up/maintenance: only K=128 N=512 matmuls feed
                # the PE activity monitor; the clock needs a dense burst
                # to reach 2.4 GHz and >=1 qualifying matmul per ~3.4us
                # window to stay there
                keep = pspool.tile([128, 512], F32, tag="ps", name="keep")
                nc.tensor.matmul(keep, ident, scratch, start=True, stop=True)

            # dense warm-up burst (~4us of back-to-back qualifying MMs)
            for _ in range(10):
                keeper()

            # attention index i = 2*p + qc; quantum deadlines:
            #   P(p) quanta must drain before attention 2p (first use)
            #   AVFIN of attention i must drain before attention i+2
            #   (its at/vh ring slots are reused there)
            qh_t, kh_t, vh_t, head0, v0 = make_pair_tiles(
                0, deadline=-1, split_v=True)
            for j, (_, _, fn) in enumerate(head0):
                fn()
                if j % 2 == 1:
                    keeper()
            deferred.extend((1, c, f) for _, c, f in v0)
            cur = (qh_t, kh_t, vh_t)
            for p in range(N_PAIR):
                if p < N_PAIR - 1:
                    nq, nk, nv, quanta = make_pair_tiles(
                        p + 1, deadline=2 * (p + 1))
                    deferred.extend(quanta)
                for qc in range(2):
                    i = 2 * p + qc
                    at0, at1 = attention(i, p, qc, *cur)
                    enqueue_avfin(i + 2, p, qc, at0, at1, cur[2])
                if p < N_PAIR - 1:
                    cur = (nq, nk, nv)

            drain_due(8)        # finish (pair 3, qc0) AV/finalize
            for qt in range(4):
                final_proj(qt)
                drain_budget(3400)   # overlap (pair 3, qc1) AV/finalize
            drain_budget(10 ** 9)
            for qt in range(4, N_QT):
                final_proj(qt)

    nc.compile()
    return nc


def _prep_inputs(q, k, v, wq_w, wq_b, wk_w, wk_b, wv_w, wv_b, wo_w, wo_b):
    """Host-side shard + layout + cast. Returns per-core input maps."""
    def bf(x):
        return np.ascontiguousarray(np.asarray(x, np.float32)).astype(BF16)

    # wo2[64*hi + d, p, :] = wo_w[(2p + hi)*64 + d, :]
    wo2 = np.ascontiguousarray(
        np.asarray(wo_w, np.float32).reshape(N_PAIR, 2, DEP, D)
        .transpose(1, 2, 0, 3).reshape(128, N_PAIR, D))
    shared = {
        "wq": bf(wq_w), "wk": bf(wk_w), "wv": bf(wv_w), "wo": bf(wo2),
        "bq": bf(wq_b).reshape(1, D), "bk": bf(wk_b).reshape(1, D),
        "bv": bf(wv_b).reshape(1, D), "bo": bf(wo_b).reshape(1, D),
    }
    kT_b = [np.ascontiguousarray(bf(k[b_]).T) for b_ in range(B)]
    vT_b = [np.ascontiguousarray(bf(v[b_]).T) for b_ in range(B)]
    in_maps = []
    for c in range(N_CORES):
        b_ = c // 4
        r0 = (c % 4) * SQ
        m = dict(shared)
        m["qT"] = np.ascontiguousarray(bf(q[b_][r0:r0 + SQ]).T)
        m["kT"] = kT_b[b_]
        m["vT"] = vT_b[b_]
        in_maps.append(m)
    return in_maps


def kernel(q, k, v, wq_w, wq_b, wk_w, wk_b, wv_w, wv_b, wo_w, wo_b,
           trace=False):
    global _COMPILED
    with_bias = any(np.any(np.asarray(b)) for b in (wq_b, wk_b, wv_b, wo_b))
    if _COMPILED is None or _COMPILED[0] != with_bias:
        _COMPILED = (with_bias, build_kernel(with_bias=with_bias))
    nc = _COMPILED[1]
    in_maps = _prep_inputs(q, k, v, wq_w, wq_b, wk_w, wk_b, wv_w, wv_b,
                           wo_w, wo_b)
    global _WARMED
    if not _WARMED:
        # first execution after a NEFF load runs ~30% slower (cold DMA
        # rings / tables); do a throwaway warmup run
        run_bass_kernel_spmd(nc, in_maps, list(range(N_CORES)), trace=False)
        _WARMED = True
    res = run_bass_kernel_spmd(nc, in_maps, list(range(N_CORES)), trace=trace)
    out = np.empty((B, S, D), np.float32)
    for c in range(N_CORES):
        b_ = c // 4
        r0 = (c % 4) * SQ
        out[b_, r0:r0 + SQ] = res.results[c]["out"]
    kernel.last_exec_time_ns = res.exec_time_ns
    return out


if __name__ == "__main__":
    rng = np.random.default_rng(0)
    ins = {
        "q": rng.normal(size=(B, S, D)).astype(np.float32),
        "k": rng.normal(size=(B, S, D)).astype(np.float32),
        "v": rng.normal(size=(B, S, D)).astype(np.float32),
    }
    sc_ = 1.0 / np.sqrt(D)
    for n in ("wq", "wk", "wv", "wo"):
        ins[n + "_w"] = (rng.normal(size=(D, D)) * sc_).astype(np.float32)
        ins[n + "_b"] = np.zeros(D, np.float32)
    o = kernel(**ins)
    print("out shape", o.shape, "mean abs", np.abs(o).mean())


# revision 29
# speedup vs baseline: 1.0230x; 1.0230x over previous
"""Multi-head attention (B=2, S=4096, D=512, H=8) on 8 Trainium2 NeuronCores.

Sharding: core c handles batch b = c//4 and q-rows [1024*(c%4), 1024*(c%4+1)).
No collectives.

Schedule (v2): the Scalar engine's softmax-exp (33.6M elem/core at 1
elem/lane/cycle @1.2GHz ~ 220us busy) is the hard floor, so everything else
is organized to hide under it:
  - head-PAIR processing: the two heads of a pair occupy partition halves
    0-63 / 64-127; their K=64 score matmuls are issued back-to-back with
    tile_position (0,0)/(64,0) so they run CONCURRENTLY in the two row
    halves of the PE array (2x effective throughput, no warmkeepers --
    the interleaved K=128 AV/projection matmuls keep the HAM clock warm).
  - projections for pair p+1 and the AV/finalize work of the previous
    (pair,qc) are chopped into small "quanta" and drained into the score
    group loop, so the tensor engine works while the scalar engine exps.
    Quanta carry a deadline (attention index) and are force-drained
    before the attention that needs their results / their ring slots.
  - PSUM map (8 banks): scores 2x[128,2,512] (4) + av 2x[128,4,128] (2,
    one per head, qt accumulation groups sequential within the bank as
    required by the 2KB zero-region rule) + proj/transpose ring
    2x[128,512] (2).
  - output transposes are col-tiled pairs (0,0)/(0,64) into one [128,128]
    PSUM tile; final projection contracts head pairs as row-tiled
    concurrent matmuls into two separate PSUM tiles (summed on DVE).

Numerics identical to v1: bf16 matmul operands, fp32 PSUM accumulation,
softmax exp in fp32->bf16 on Scalar (no max-subtraction; scores ~N(0,1)),
denominator via ones-column in the attn@V matmul, normalization on DVE.
"""

import numpy as np
import ml_dtypes

import concourse.bass as bass
import concourse.tile as tile
import concourse.mybir as mybir
from concourse import bacc
from concourse.bass_utils import run_bass_kernel_spmd
from concourse.masks import make_identity

BF16 = ml_dtypes.bfloat16
F32 = mybir.dt.float32
BF = mybir.dt.bfloat16
EXP = mybir.ActivationFunctionType.Exp

N_CORES = 8
B, S, D = 2, 4096, 512
H, DEP = 8, 64
SQ = S // 4            # q rows per core
N_QT = SQ // 128       # q 128-tiles per core (8)
N_KT = S // 128        # k 128-tiles (32)
N_DC = D // 128        # 128-chunks of d_model (4)
N_PAIR = 4             # head pairs; pair p = heads 2p, 2p+1
G = 2                  # k-tiles per score PSUM tile / exp instruction

_COMPILED = None
_WARMED = False


def build_kernel(with_bias=True):
    nc = bacc.Bacc("TRN2", target_bir_lowering=False, debug=False,
                   num_devices=N_CORES)

    # ---- I/O ----
    qT = nc.dram_tensor("qT", [D, SQ], BF, kind="ExternalInput")
    kT = nc.dram_tensor("kT", [D, S], BF, kind="ExternalInput")
    vT = nc.dram_tensor("vT", [D, S], BF, kind="ExternalInput")
    w_in, b_in = {}, {}
    for name in ("wq", "wk", "wv"):
        w_in[name] = nc.dram_tensor(name, [D, D], BF, kind="ExternalInput")
    # wo host-reorganized as [128, N_PAIR, D]: rows 0:64 = head 2p's 64
    # contraction dims, rows 64:128 = head 2p+1's
    wo_in = nc.dram_tensor("wo", [128, N_PAIR, D], BF, kind="ExternalInput")
    for name in ("bq", "bk", "bv", "bo"):
        b_in[name] = nc.dram_tensor(name, [1, D], BF, kind="ExternalInput")
    out = nc.dram_tensor("out", [SQ, D], F32, kind="ExternalOutput")

    with tile.TileContext(nc) as tc:
        with (
            tc.tile_pool(name="const", bufs=1) as cpool,
            tc.tile_pool(name="hring", bufs=2) as hpool,
            tc.tile_pool(name="atp", bufs=2) as atpool,
            tc.tile_pool(name="xin", bufs=2) as xpool,
            tc.tile_pool(name="small", bufs=4) as spool,
            tc.tile_pool(name="big", bufs=1) as bigpool,
            tc.tile_pool(name="scores", bufs=2, space="PSUM") as scpool,
            tc.tile_pool(name="avps", bufs=2, space="PSUM") as avpool,
            tc.tile_pool(name="projps", bufs=2, space="PSUM") as pspool,
        ):
            # ---- constants ----
            ident = cpool.tile([128, 128], BF, name="ident")
            make_identity(nc, ident)
            scratch = cpool.tile([128, 512], BF, name="scratch")
            nc.gpsimd.memset(scratch, 0.25)

            wsb, bsb = {}, {}
            for name in ("wq", "wk", "wv"):
                t = cpool.tile([128, N_DC, D], BF, name=f"w_{name}")
                for c in range(N_DC):
                    nc.sync.dma_start(t[:, c, :],
                                      w_in[name][c * 128:(c + 1) * 128, :])
                wsb[name] = t
            wosb = cpool.tile([128, N_PAIR, D], BF, name="w_wo")
            nc.sync.dma_start(wosb, wo_in[:])
            if with_bias:
                ones = cpool.tile([1, 512], BF, name="ones")
                nc.gpsimd.memset(ones, 1.0)
                btile = cpool.tile([1, 4, D], BF, name="biases")
                for i, name in enumerate(("bq", "bk", "bv", "bo")):
                    nc.sync.dma_start(btile[:, i, :], b_in[name][:])
                    bsb[name] = btile[:, i, :]

            # transposed per-head attn outputs, heads paired on partition
            # halves: otr2[64*hi:64*(hi+1), qt, p, :] = head (2p+hi)
            otr2 = bigpool.tile([128, N_QT, N_PAIR, 128], BF, name="otr2")

            # ---- deferred work queue: (deadline_att_idx, cost, fn) ----
            deferred = []
            oh_live = {}

            def drain_budget(budget):
                spent = 0
                while deferred and spent < budget:
                    _, cost, fn = deferred.pop(0)
                    fn()
                    spent += cost

            def drain_due(att_idx):
                while deferred and deferred[0][0] <= att_idx:
                    _, _, fn = deferred.pop(0)
                    fn()

            # ---- projection quanta for pair p ----
            # each quantum is split into a DMA-issue part and an MM part;
            # the queue interleaves them two-deep (ring bufs=2) so the PE
            # stream never waits on an xin load mid-queue
            def qk_quantum(src, wname, bname, p, rc, dst):
                box = {}

                def dma_fn():
                    xin = xpool.tile([128, N_DC, 512], BF, tag="xin",
                                     name="xin")
                    for dc in range(N_DC):
                        nc.sync.dma_start(
                            xin[:, dc, :],
                            src[dc * 128:(dc + 1) * 128,
                                rc * 512:(rc + 1) * 512])
                    box["xin"] = xin

                def mm_fn():
                    xin = box["xin"]
                    ps = pspool.tile([128, 512], F32, tag="ps", name="ps")
                    for dc in range(N_DC):
                        nc.tensor.matmul(
                            ps, wsb[wname][:, dc, p * 128:(p + 1) * 128],
                            xin[:, dc, :], start=(dc == 0),
                            stop=(not with_bias and dc == N_DC - 1))
                    if with_bias:
                        nc.tensor.matmul(
                            ps, bsb[bname][0:1, p * 128:(p + 1) * 128],
                            ones[0:1, :], start=False, stop=True)
                    nc.vector.tensor_copy(dst[:, rc * 512:(rc + 1) * 512],
                                          ps)
                return (2300, dma_fn, mm_fn)

            def v_quantum(p, rc, half, vh_t):
                # projects seq rows [rc*512 + half*256, +256) = 2 k-tiles,
                # one PSUM accumulation group spanning both 128-col slices
                box = {}
                c0 = rc * 512 + half * 256

                def dma_fn():
                    xin = xpool.tile([128, N_DC, 256], BF, tag="xin",
                                     name="xinv")
                    for dc in range(N_DC):
                        nc.sync.dma_start(
                            xin[:, dc, :],
                            vT[dc * 128:(dc + 1) * 128, c0:c0 + 256])
                    box["xin"] = xin

                def mm_fn():
                    xin = box["xin"]
                    ps = pspool.tile([128, 512], F32, tag="ps", name="psv")
                    nmm = 2 * N_DC + (2 if with_bias else 0)
                    j = 0
                    for i in range(2):
                        for dc in range(N_DC):
                            nc.tensor.matmul(
                                ps[:, i * 128:(i + 1) * 128],
                                xin[:, dc, i * 128:(i + 1) * 128],
                                wsb["wv"][:, dc, p * 128:(p + 1) * 128],
                                start=(j == 0), stop=(j == nmm - 1))
                            j += 1
                        if with_bias:
                            nc.tensor.matmul(
                                ps[:, i * 128:(i + 1) * 128],
                                ones[0:1, 0:128],
                                bsb["bv"][0:1, p * 128:(p + 1) * 128],
                                start=False, stop=(j == nmm - 1))
                            j += 1
                    rt0 = c0 // 128
                    nc.vector.tensor_copy(
                        vh_t[:, rt0:rt0 + 2, :, 0:DEP],
                        ps[:, 0:256].rearrange("p (r h e) -> p r h e",
                                               r=2, h=2))
                return (1300, dma_fn, mm_fn)

            def q_quantum(p, rc, qhz_t):
                # like qk_quantum but writes the two zero-padded per-head
                # copies: qhz[:, hi, :] has the OTHER head's 64 partitions
                # zeroed so the score matmul can contract over the full
                # K=128 (feeding the HAM clock monitor) yet still compute
                # a single head's scores
                box = {}

                def dma_fn():
                    xin = xpool.tile([128, N_DC, 512], BF, tag="xin",
                                     name="xin")
                    for dc in range(N_DC):
                        nc.sync.dma_start(
                            xin[:, dc, :],
                            qT[dc * 128:(dc + 1) * 128,
                               rc * 512:(rc + 1) * 512])
                    box["xin"] = xin

                def mm_fn():
                    xin = box["xin"]
                    ps = pspool.tile([128, 512], F32, tag="ps", name="ps")
                    for dc in range(N_DC):
                        nc.tensor.matmul(
                            ps, wsb["wq"][:, dc, p * 128:(p + 1) * 128],
                            xin[:, dc, :], start=(dc == 0),
                            stop=(not with_bias and dc == N_DC - 1))
                    if with_bias:
                        nc.tensor.matmul(
                            ps, bsb["bq"][0:1, p * 128:(p + 1) * 128],
                            ones[0:1, :], start=False, stop=True)
                    sl = slice(rc * 512, (rc + 1) * 512)
                    nc.vector.tensor_copy(qhz_t[0:64, 0, sl], ps[0:64, :])
                    nc.vector.tensor_copy(qhz_t[64:128, 1, sl],
                                          ps[64:128, :])
                return (2300, dma_fn, mm_fn)

            def stagger(parts, deadline):
                # entry j runs mm_{j-1} then issues dma_j, so the PE work
                # of one quantum overlaps the next quantum's loads
                quanta = [(deadline, 100, parts[0][1])]
                for j in range(1, len(parts)):
                    dma_j, mm_prev = parts[j][1], parts[j - 1][2]

                    def entry(mm=mm_prev, dma=dma_j):
                        mm()
                        dma()
                    quanta.append((deadline, parts[j - 1][0], entry))
                quanta.append((deadline, parts[-1][0], parts[-1][2]))
                return quanta

            def make_pair_tiles(p, deadline, split_v=False):
                qhz_t = hpool.tile([128, 2, SQ], BF, tag="qh", name="qhz")
                kh_t = hpool.tile([128, S], BF, tag="kh", name="kh")
                vh_t = hpool.tile([128, N_KT, 2, DEP + 1], BF, tag="vh",
                                  name="vh")

                def memset_fn():
                    nc.gpsimd.memset(vh_t[:, :, :, DEP:DEP + 1], 1.0)
                    nc.gpsimd.memset(qhz_t[64:128, 0, :], 0.0)
                    nc.gpsimd.memset(qhz_t[0:64, 1, :], 0.0)
                if split_v:
                    # pair 0: q/k quanta run inline so attention can start
                    # early; v quanta drain during the first windows
                    qk_parts = [q_quantum(p, rc, qhz_t)
                                for rc in range(SQ // 512)]
                    qk_parts += [qk_quantum(kT, "wk", "bk", p, rc, kh_t)
                                 for rc in range(S // 512)]
                    v_parts = []
                    for rc in range(S // 512):
                        v_parts.append(v_quantum(p, rc, 0, vh_t))
                        v_parts.append(v_quantum(p, rc, 1, vh_t))
                    head = ([(deadline, 100, memset_fn)]
                            + stagger(qk_parts, deadline))
                    return qhz_t, kh_t, vh_t, head, stagger(v_parts,
                                                            deadline)
                parts = []
                for rc in range(SQ // 512):
                    parts.append(q_quantum(p, rc, qhz_t))
                for rc in range(S // 512):
                    parts.append(qk_quantum(kT, "wk", "bk", p, rc, kh_t))
                    parts.append(v_quantum(p, rc, 0, vh_t))
                    parts.append(v_quantum(p, rc, 1, vh_t))
                quanta = ([(deadline, 100, memset_fn)]
                          + stagger(parts, deadline))
                return qhz_t, kh_t, vh_t, quanta

            # ---- attention for (pair p, q-chunk qc) ----
            def attention(att_idx, p, qc, qhz_t, kh_t, vh_t):
                drain_due(att_idx)
                qsl = slice(qc * 512, (qc + 1) * 512)
                at0 = atpool.tile([128, N_KT, 512], BF, tag="at0",
                                  name="at0")
                at1 = atpool.tile([128, N_KT, 512], BF, tag="at1",
                                  name="at1")
                for g in range(N_KT // G):
                    sc0 = scpool.tile([128, G, 512], F32, tag="sc",
                                      name="sc0")
                    sc1 = scpool.tile([128, G, 512], F32, tag="sc",
                                      name="sc1")
                    # full K=128 contraction: the other head's partitions
                    # are zero in qhz, so each matmul computes one head's
                    # scores while feeding the HAM clock monitor
                    for i in range(G):
                        t = g * G + i
                        nc.tensor.matmul(
                            sc0[:, i, :],
                            kh_t[:, t * 128:(t + 1) * 128],
                            qhz_t[:, 0, qsl],
                            start=True, stop=True)
                        nc.tensor.matmul(
                            sc1[:, i, :],
                            kh_t[:, t * 128:(t + 1) * 128],
                            qhz_t[:, 1, qsl],
                            start=True, stop=True)
                    nc.scalar.activation(at0[:, g * G:(g + 1) * G, :], sc0,
                                         EXP, scale=0.125)
                    nc.scalar.activation(at1[:, g * G:(g + 1) * G, :], sc1,
                                         EXP, scale=0.125)
                    drain_budget(3400)
                return at0, at1

            # ---- AV + finalize quanta for (p, qc) ----
            def avfin_quantum(hi, qt, at_h, vh_t, av_h):
                def fn():
                    for t in range(N_KT):
                        nc.tensor.matmul(
                            av_h[:, qt, 0:DEP + 1],
                            at_h[:, t, qt * 128:(qt + 1) * 128],
                            vh_t[:, t, hi, :],
                            start=(t == 0), stop=(t == N_KT - 1))
                    rec = spool.tile([128, 1], F32, tag="rec", name="rec",
                                     bufs=4)
                    nc.vector.reciprocal(rec, av_h[:, qt, DEP:DEP + 1])
                    oh = spool.tile([128, DEP], BF, tag=f"oh{hi}",
                                    name="oh", bufs=4)
                    nc.vector.tensor_scalar_mul(oh, av_h[:, qt, 0:DEP], rec)
                    oh_live[(hi, qt)] = oh
                return (2900, fn)

            def transpose_quantum(p, qc, qt):
                def fn():
                    qidx = qc * 4 + qt
                    trp = pspool.tile([128, 128], BF, tag="ps", name="trp")
                    nc.tensor.transpose(trp[0:64, :], oh_live.pop((0, qt)),
                                        ident, tile_position=(0, 0))
                    nc.tensor.transpose(trp[64:128, :], oh_live.pop((1, qt)),
                                        ident, tile_position=(0, 64))
                    nc.vector.tensor_copy(otr2[:, qidx, p, :], trp)
                return (400, fn)

            def enqueue_avfin(deadline, p, qc, at0, at1, vh_t):
                av0 = avpool.tile([128, 4, 128], F32, tag="av", name="av0")
                av1 = avpool.tile([128, 4, 128], F32, tag="av", name="av1")
                for hi, (at_h, av_h) in enumerate(((at0, av0), (at1, av1))):
                    for qt in range(4):
                        deferred.append(
                            (deadline,)
                            + avfin_quantum(hi, qt, at_h, vh_t, av_h))
                for qt in range(4):
                    deferred.append((deadline,) + transpose_quantum(p, qc, qt))

            # ---- final output projection for one q-tile ----
            # full K=128 contraction per pair sums both heads' (d=64)
            # contributions in one matmul
            def final_proj(qt):
                ps = pspool.tile([128, 512], F32, tag="ps", name="pso")
                for s in range(N_PAIR):
                    nc.tensor.matmul(
                        ps, otr2[:, qt, s, :], wosb[:, s, :],
                        start=(s == 0),
                        stop=(not with_bias and s == N_PAIR - 1))
                if with_bias:
                    nc.tensor.matmul(ps, ones[0:1, 0:128], bsb["bo"],
                                     start=False, stop=True)
                osb = spool.tile([128, 512], F32, tag="osb", name="osb",
                                 bufs=2)
                nc.vector.tensor_copy(osb, ps)
                nc.sync.dma_start(out[qt * 128:(qt + 1) * 128, :], osb)

            # ---- main flow ----
            def keeper():
                # HAM warm-up/maintenance: only K=128 N=512 matmuls feed
                # the PE activity monitor; the clock needs a dense burst
                # to reach 2.4 GHz and >=1 qualifying matmul per ~3.4us
                # window to stay there
                keep = pspool.tile([128, 512], F32, tag="ps", name="keep")
                nc.tensor.matmul(keep, ident, scratch, start=True, stop=True)

            # dense warm-up burst (~4us of back-to-back qualifying MMs)
            for _ in range(10):
                keeper()

            # attention index i = 2*p + qc; quantum deadlines:
            #   P(p) quanta must drain before attention 2p (first use)
            #   AVFIN of attention i must drain before attention i+2
            #   (its at/vh ring slots are reused there)
            qh_t, kh_t, vh_t, quanta0 = make_pair_tiles(0, deadline=-1)
            for j, (_, _, fn) in enumerate(quanta0):
                fn()
                if j % 2 == 1:
                    keeper()
            cur = (qh_t, kh_t, vh_t)
            for p in range(N_PAIR):
                if p < N_PAIR - 1:
                    nq, nk, nv, quanta = make_pair_tiles(
                        p + 1, deadline=2 * (p + 1))
                    deferred.extend(quanta)
                for qc in range(2):
                    i = 2 * p + qc
                    at0, at1 = attention(i, p, qc, *cur)
                    enqueue_avfin(i + 2, p, qc, at0, at1, cur[2])
                if p < N_PAIR - 1:
                    cur = (nq, nk, nv)

            drain_due(8)        # finish (pair 3, qc0) AV/finalize
            for qt in range(4):
                final_proj(qt)
                drain_budget(3400)   # overlap (pair 3, qc1) AV/finalize
            drain_budget(10 ** 9)
            for qt in range(4, N_QT):
                final_proj(qt)

    nc.compile()
    return nc


def _prep_inputs(q, k, v, wq_w, wq_b, wk_w, wk_b, wv_w, wv_b, wo_w, wo_b):
    """Host-side shard + layout + cast. Returns per-core input maps."""
    def bf(x):
        return np.ascontiguousarray(np.asarray(x, np.float32)).astype(BF16)

    # wo2[64*hi + d, p, :] = wo_w[(2p + hi)*64 + d, :]
    wo2 = np.ascontiguousarray(
        np.asarray(wo_w, np.float32).reshape(N_PAIR, 2, DEP, D)
        .transpose(1, 2, 0, 3).reshape(128, N_PAIR, D))
    shared = {
        "wq": bf(wq_w), "wk": bf(wk_w), "wv": bf(wv_w), "wo": bf(wo2),
        "bq": bf(wq_b).reshape(1, D), "bk": bf(wk_b).reshape(1, D),
        "bv": bf(wv_b).reshape(1, D), "bo": bf(wo_b).reshape(1, D),
    }
    kT_b = [np.ascontiguousarray(bf(k[b_]).T) for b_ in range(B)]
    vT_b = [np.ascontiguousarray(bf(v[b_]).T) for b_ in range(B)]
    in_maps = []
    for c in range(N_CORES):
        b_ = c // 4
        r0 = (c % 4) * SQ
        m = dict(shared)
        m["qT"] = np.ascontiguousarray(bf(q[b_][r0:r0 + SQ]).T)
        m["kT"] = kT_b[b_]
        m["vT"] = vT_b[b_]
        in_maps.append(m)
    return in_maps


def kernel(q, k, v, wq_w, wq_b, wk_w, wk_b, wv_w, wv_b, wo_w, wo_b,
           trace=False):
    global _COMPILED
    with_bias = any(np.any(np.asarray(b)) for b in (wq_b, wk_b, wv_b, wo_b))
    if _COMPILED is None or _COMPILED[0] != with_bias:
        _COMPILED = (with_bias, build_kernel(with_bias=with_bias))
    nc = _COMPILED[1]
    in_maps = _prep_inputs(q, k, v, wq_w, wq_b, wk_w, wk_b, wv_w, wv_b,
                           wo_w, wo_b)
    global _WARMED
    if not _WARMED:
        # first execution after a NEFF load runs ~30% slower (cold DMA
        # rings / tables); do a throwaway warmup run
        run_bass_kernel_spmd(nc, in_maps, list(range(N_CORES)), trace=False)
        _WARMED = True
    res = run_bass_kernel_spmd(nc, in_maps, list(range(N_CORES)), trace=trace)
    out = np.empty((B, S, D), np.float32)
    for c in range(N_CORES):
        b_ = c // 4
        r0 = (c % 4) * SQ
        out[b_, r0:r0 + SQ] = res.results[c]["out"]
    kernel.last_exec_time_ns = res.exec_time_ns
    return out


if __name__ == "__main__":
    rng = np.random.default_rng(0)
    ins = {
        "q": rng.normal(size=(B, S, D)).astype(np.float32),
        "k": rng.normal(size=(B, S, D)).astype(np.float32),
        "v": rng.normal(size=(B, S, D)).astype(np.float32),
    }
    sc_ = 1.0 / np.sqrt(D)
    for n in ("wq", "wk", "wv", "wo"):
        ins[n + "_w"] = (rng.normal(size=(D, D)) * sc_).astype(np.float32)
        ins[n + "_b"] = np.zeros(D, np.float32)
    o = kernel(**ins)
    print("out shape", o.shape, "mean abs", np.abs(o).mean())
